# revision 22
# baseline (speedup 1.0000x reference)
"""DiffAttention TRN2 kernel: 8-way (batch x seq-half) sharded, zero collectives.

v2: all-bf16 matmuls, 2x row-tiled S (tile_position), exp split ACT/DVE
(Schraudolph bit-trick on DVE), division-free combine (od2 = o1*d2 - lam*o2*d1)
with batched end-of-loop normalization rows.

Shapes: x [4, 4096, 1024], H=16 heads, head = (h, 2 branches, 32 dims),
v head dim 64. Each core: one (batch, query-half) = 2048 query rows,
recomputes K/V for its batch's full 4096 keys.

Pipeline per (head, branch, qc-half, kt):
  S^T tile [128 keys, 1024 q] in PSUM from two concurrent row-tiled bf16
  matmuls (lhsT = K slice [32,128] at partitions 0/64); exp on ACT (fp32->bf16,
  scale folded) or Schraudolph on DVE (int16 out, bitcast bf16); PV accumulates
  o_ps [65, 1024] with augmented V (65th col ones -> softmax denominators).
"""

import sys

import numpy as np
import ml_dtypes

for p in ("/opt/trn_rl_repo",):
    if p not in sys.path:
        sys.path.insert(0, p)

import concourse.bass as bass
import concourse.bacc as bacc_mod
import concourse.mybir as mybir
from concourse.bass_utils import run_bass_kernel_spmd
from concourse.tile import TileContext

F32 = mybir.dt.float32
BF = mybir.dt.bfloat16
I16 = mybir.dt.int16

B, N, DIM, H, HD = 4, 4096, 1024, 16, 32
VD = 2 * HD  # 64, per-head v dim
NQ = 2048  # query rows per core
KT = N // 128  # 32 key tiles
CIN = DIM // 128  # 8 contraction tiles
NCORES = 8
LAMBDA_INIT = 0.2
EPS = 1e-5
SCALE = HD ** -0.5

# Schraudolph exp on DVE: i16 = round(s*SCALE*128*log2e + (127-sigma)*128),
# bitcast bf16 ~= exp(s*SCALE). sigma=0.043 minimizes max rel err (~3.3%).
A_DVE = float(SCALE * 128.0 / np.log(2.0))
B_DVE = float((127.0 - 0.043) * 128.0)
DVE_EXP_MOD = 3  # kt % 3 == 2 -> DVE (1/3 of exp on DVE)

_CACHE = {}


def build_nc(lam: float):
    nc = bacc_mod.Bacc(None, target_bir_lowering=False)
    AF = mybir.ActivationFunctionType
    ALU = mybir.AluOpType

    xbt = nc.declare_dram_parameter("xbt", [DIM, N], BF, isOutput=False)
    wqkvt = nc.declare_dram_parameter("wqkvt", [DIM, 3 * DIM], BF, isOutput=False)
    wprojt = nc.declare_dram_parameter("wprojt", [DIM, DIM], BF, isOutput=False)
    bprojr = nc.declare_dram_parameter("bprojr", [1, DIM], BF, isOutput=False)
    weffr = nc.declare_dram_parameter("weffr", [1, VD], BF, isOutput=False)
    y = nc.declare_dram_parameter("y", [NQ, DIM], F32, isOutput=True)

    qt_s = nc.dram_tensor("qt_scratch", [DIM, NQ], BF)
    kt_s = nc.dram_tensor("kt_scratch", [DIM, N], BF)
    v_s = nc.dram_tensor("v_scratch", [N, DIM], BF)
    od_s = nc.dram_tensor("od_scratch", [H, VD, NQ], BF)

    with nc.allow_low_precision(reason="bf16 verified 5e-3 rel vs 2e-2 tol"), \
         TileContext(nc) as tc:
        with tc.tile_pool(name="const", bufs=1) as constp:
            # constant rows live at partition 64 (d-rows of ov sit there)
            ones_w65 = constp.tile([65, VD], BF)
            nc.vector.memset(ones_w65, 1.0)
            ones_w = ones_w65[64:65, :]
            lam_w65 = constp.tile([65, VD], BF)
            nc.vector.memset(lam_w65, lam)
            lam_w = lam_w65[64:65, :]
            ones64c = constp.tile([VD, 1], BF)
            nc.vector.memset(ones64c, 1.0)
            ones1 = constp.tile([1, 128], BF)
            nc.vector.memset(ones1, 1.0)
            eps_t = constp.tile([16, 1], F32)
            nc.vector.memset(eps_t, EPS)
            # weff row at partition 64 (rr rows are DMA'd there per head)
            weff_w65 = constp.tile([65, VD], BF)
            nc.sync.dma_start(out=weff_w65[64:65, :], in_=weffr[:, :])

            # ================= phase A: qkv =================
            drain_idx = 0

            def drain(dst_ap, src_ps):
                nonlocal drain_idx
                if drain_idx % 2 == 0:
                    nc.vector.tensor_copy(dst_ap, src_ps)
                else:
                    nc.scalar.activation(dst_ap, src_ps, AF.Copy)
                drain_idx += 1

            with (
                tc.tile_pool(name="xbt_p", bufs=1) as xbtp,
                tc.tile_pool(name="wq_p", bufs=4) as wqp,
                tc.tile_pool(name="psA", bufs=3, space="PSUM") as psA,
                tc.tile_pool(name="psV", bufs=2, space="PSUM") as psV,
                tc.tile_pool(name="drain_p", bufs=3) as drp,
            ):
                xb = xbtp.tile([128, CIN, N], BF)
                nc.sync.dma_start(
                    out=xb, in_=xbt[:, :].rearrange("(t p) n -> p t n", p=128)
                )
                # --- Q^T and K^T co-tiles ---
                for co in range(2 * CIN):  # 0..7 Q, 8..15 K
                    is_q = co < CIN
                    tok = NQ if is_q else N
                    for ch in range(tok // 1024):
                        ps = psA.tile([128, 1024], F32, tag="ps")
                        for ci in range(CIN):
                            wt = wqp.tile([128, 128], BF, tag="w")
                            nc.sync.dma_start(
                                out=wt,
                                in_=wqkvt[ci * 128:(ci + 1) * 128,
                                          co * 128:(co + 1) * 128],
                            )
                            for sb in range(2):
                                nc.tensor.matmul(
                                    ps[:, sb * 512:(sb + 1) * 512],
                                    wt,
                                    xb[:, ci, ch * 1024 + sb * 512:
                                       ch * 1024 + (sb + 1) * 512],
                                    start=(ci == 0),
                                    stop=(ci == CIN - 1),
                                )
                        dr = drp.tile([128, 1024], BF, tag="dr")
                        drain(dr, ps)
                        dst = qt_s if is_q else kt_s
                        coo = co if is_q else co - CIN
                        nc.sync.dma_start(
                            out=dst[coo * 128:(coo + 1) * 128,
                                    ch * 1024:(ch + 1) * 1024],
                            in_=dr,
                        )
                # --- V (untransposed) in c-chunks of 256 ---
                with tc.tile_pool(name="wv_p", bufs=8) as wvp:
                    for cc in range(DIM // 256):
                        wv_tiles = []
                        for ci in range(CIN):
                            wv = wvp.tile([128, 256], BF, tag="wv")
                            nc.sync.dma_start(
                                out=wv,
                                in_=wqkvt[ci * 128:(ci + 1) * 128,
                                          2 * DIM + cc * 256:
                                          2 * DIM + (cc + 1) * 256],
                            )
                            wv_tiles.append(wv)
                        for kt in range(KT):
                            psv = psV.tile([128, 256], F32, tag="psv")
                            for ci in range(CIN):
                                nc.tensor.matmul(
                                    psv,
                                    xb[:, ci, kt * 128:(kt + 1) * 128],
                                    wv_tiles[ci],
                                    start=(ci == 0),
                                    stop=(ci == CIN - 1),
                                )
                            drv = drp.tile([128, 256], BF, tag="drv")
                            drain(drv, psv)
                            nc.sync.dma_start(
                                out=v_s[kt * 128:(kt + 1) * 128,
                                        cc * 256:(cc + 1) * 256],
                                in_=drv,
                            )

            # ================= phase B: attention =================
            with tc.tile_pool(name="coll_p", bufs=1) as collp:
                dd1c = collp.tile([H, NQ], BF)
                dd2c = collp.tile([H, NQ], BF)
                msrc = collp.tile([H, NQ], F32)
                ot_acc = collp.tile([128, CIN, NQ], BF)

                with (
                    tc.tile_pool(name="psS", bufs=3, space="PSUM") as psS,
                    tc.tile_pool(name="psO", bufs=1, space="PSUM") as psO,
                    tc.tile_pool(name="kh_p", bufs=2) as khp,
                    tc.tile_pool(name="qh_p", bufs=2) as qhp,
                    tc.tile_pool(name="vh_p", bufs=2) as vhp,
                    tc.tile_pool(name="es_p", bufs=6) as esp,
                    tc.tile_pool(name="ov_p", bufs=3) as ovp,
                    tc.tile_pool(name="cmb_p", bufs=2) as cmbp,
                    tc.tile_pool(name="row_p", bufs=2) as rowp,
                ):
                    for h in range(H):
                        ovs = [None, None]
                        for br in range(2):
                            r0 = h * VD + br * HD
                            kh = khp.tile([128, KT, 128], BF, tag="kh")
                            qh = qhp.tile([128, NQ], BF, tag="qh")
                            for bp in (0, 64):
                                nc.sync.dma_start(
                                    out=kh[bp:bp + HD],
                                    in_=kt_s[r0:r0 + HD, :].rearrange(
                                        "d (t x) -> d t x", x=128),
                                )
                                nc.sync.dma_start(
                                    out=qh[bp:bp + HD], in_=qt_s[r0:r0 + HD, :]
                                )
                            if br == 0:
                                vh = vhp.tile([128, KT, 65], BF, tag="vh")
                                nc.vector.memset(vh[:, :, VD:65], 1.0)
                                nc.sync.dma_start(
                                    out=vh[:, :, 0:VD],
                                    in_=v_s[:, h * VD:(h + 1) * VD].rearrange(
                                        "(t p) c -> p t c", p=128),
                                )
                            ov = ovp.tile([65, NQ], BF, tag="ov")
                            ovs[br] = ov
                            for qch in range(2):
                                o_ps = psO.tile([65, 1024], F32, tag="o")
                                q0 = qch * 1024
                                # software pipeline: S/exp run 2 units ahead of
                                # PV so a PV waiting on exp never blocks an S
                                # in the strict-FIFO PE queue.
                                es_q = {}

                                def emit_s_exp(kt):
                                    sps = psS.tile([128, 1024], F32, tag="s")
                                    for half in range(2):
                                        bp = 64 * half
                                        nc.tensor.matmul(
                                            sps[:, half * 512:(half + 1) * 512],
                                            kh[bp:bp + HD, kt, :],
                                            qh[bp:bp + HD,
                                               q0 + half * 512:
                                               q0 + (half + 1) * 512],
                                            start=True,
                                            stop=True,
                                            tile_position=(bp, 0),
                                        )
                                    if kt % DVE_EXP_MOD == DVE_EXP_MOD - 1:
                                        esi = esp.tile([128, 1024], I16, tag="es")
                                        nc.vector.tensor_scalar(
                                            esi, sps, A_DVE, B_DVE,
                                            op0=ALU.mult, op1=ALU.add,
                                        )
                                        es_q[kt] = esi.bitcast(BF)
                                    else:
                                        es = esp.tile([128, 1024], BF, tag="es")
                                        nc.scalar.activation(
                                            es, sps, AF.Exp, scale=SCALE
                                        )
                                        es_q[kt] = es

                                def emit_pv(kt):
                                    es_ = es_q.pop(kt)
                                    for half in range(2):
                                        nc.tensor.matmul(
                                            o_ps[:, half * 512:(half + 1) * 512],
                                            vh[:, kt, :],
                                            es_[:, half * 512:(half + 1) * 512],
                                            start=(kt == 0),
                                            stop=(kt == KT - 1),
                                        )

                                for kt in range(KT):
                                    emit_s_exp(kt)
                                    if kt >= 2:
                                        emit_pv(kt - 2)
                                emit_pv(KT - 2)
                                emit_pv(KT - 1)
                                nc.vector.tensor_copy(ov[:, q0:q0 + 1024], o_ps)
                        # ---- combine head h: od2 = o1*d2 - lam*o2*d1 ----
                        ov0, ov1 = ovs
                        od2f = cmbp.tile([VD, NQ], BF, tag="od2")
                        for qch in range(2):
                            sl = slice(qch * 1024, qch * 1024 + 1024)
                            bc2 = psS.tile([VD, 1024], F32, tag="s")
                            bc1 = psS.tile([VD, 1024], F32, tag="s")
                            for sb in range(2):
                                s2 = slice(qch * 1024 + sb * 512,
                                           qch * 1024 + (sb + 1) * 512)
                                so = slice(sb * 512, (sb + 1) * 512)
                                nc.tensor.matmul(
                                    bc2[:, so], ones_w, ov1[64:65, s2],
                                    start=True, stop=True,
                                )
                                nc.tensor.matmul(
                                    bc1[:, so], lam_w, ov0[64:65, s2],
                                    start=True, stop=True,
                                )
                            m1 = cmbp.tile([VD, 1024], BF, tag="m1")
                            nc.vector.tensor_mul(m1, ov0[0:VD, sl], bc2)
                            m2 = cmbp.tile([VD, 1024], BF, tag="m2")
                            nc.vector.tensor_mul(m2, ov1[0:VD, sl], bc1)
                            nc.vector.tensor_sub(od2f[:, sl], m1, m2)
                            sq = cmbp.tile([VD, 1024], BF, tag="sq")
                            nc.vector.tensor_mul(sq, od2f[:, sl], od2f[:, sl])
                            msr_ps = psS.tile([1, 1024], F32, tag="s")
                            for sb in range(2):
                                so = slice(sb * 512, (sb + 1) * 512)
                                nc.tensor.matmul(
                                    msr_ps[:, so], ones64c, sq[:, so],
                                    start=True, stop=True,
                                )
                            msr_t = rowp.tile([1, 1024], F32, tag="row")
                            nc.vector.tensor_copy(msr_t, msr_ps)
                            nc.sync.dma_start(out=msrc[h:h + 1, sl], in_=msr_t)
                        nc.sync.dma_start(out=od_s[h], in_=od2f)
                        nc.sync.dma_start(out=dd1c[h:h + 1, :], in_=ov0[64:65, :])
                        nc.sync.dma_start(out=dd2c[h:h + 1, :], in_=ov1[64:65, :])

                    # ---- batched normalization rows ----
                    with tc.tile_pool(name="bt_p", bufs=1) as btp:
                        dd = btp.tile([H, NQ], F32)
                        nc.vector.tensor_mul(dd, dd1c, dd2c)
                        idd = btp.tile([H, NQ], F32)
                        nc.vector.reciprocal(idd, dd)
                        t1 = btp.tile([H, NQ], F32)
                        nc.vector.tensor_mul(t1, msrc, idd)
                        nc.vector.tensor_mul(t1, t1, idd)
                        # sd -> reuse idd slot (idd dead after this point)
                        nc.scalar.activation(
                            idd, t1, AF.Sqrt, bias=eps_t, scale=1.0 / VD
                        )
                        nc.vector.tensor_mul(t1, idd, dd)  # sd*dd
                        rrb = btp.tile([H, NQ], BF)
                        nc.vector.reciprocal(rrb, t1)

                        # ---- apply rr * weff per head into ot_acc ----
                        for h in range(H):
                            odl = cmbp.tile([VD, NQ], BF, tag="odl")
                            nc.sync.dma_start(out=odl, in_=od_s[h])
                            rr_t = rowp.tile([65, NQ], BF, tag="rr")
                            nc.sync.dma_start(
                                out=rr_t[64:65, :], in_=rrb[h:h + 1, :]
                            )
                            ci, p0 = h // 2, (h % 2) * VD
                            if p0 == 0:
                                dst = ot_acc[0:VD, ci, :]
                            else:
                                dst = cmbp.tile([VD, NQ], BF, tag="odt")
                            for qch in range(2):
                                sl = slice(qch * 1024, qch * 1024 + 1024)
                                bcr = psS.tile([VD, 1024], F32, tag="s")
                                for sb in range(2):
                                    s2 = slice(qch * 1024 + sb * 512,
                                               qch * 1024 + (sb + 1) * 512)
                                    so = slice(sb * 512, (sb + 1) * 512)
                                    nc.tensor.matmul(
                                        bcr[:, so], weff_w65[64:65, :],
                                        rr_t[64:65, s2],
                                        start=True, stop=True,
                                    )
                                nc.vector.tensor_mul(
                                    dst[:, sl], odl[:, sl], bcr
                                )
                            if p0 != 0:
                                nc.sync.dma_start(
                                    out=ot_acc[VD:128, ci, :], in_=dst
                                )

                # ================= phase C: proj =================
                with (
                    tc.tile_pool(name="psC", bufs=2, space="PSUM") as psC,
                    tc.tile_pool(name="wp_p", bufs=1) as wpp,
                    tc.tile_pool(name="yd_p", bufs=3) as ydp,
                ):
                    wp = wpp.tile([128, CIN, DIM], BF)
                    nc.sync.dma_start(
                        out=wp,
                        in_=wprojt[:, :].rearrange("(t p) n -> p t n", p=128),
                    )
                    bp_t = wpp.tile([1, DIM], BF)
                    nc.sync.dma_start(out=bp_t, in_=bprojr[:, :])
                    for qt in range(NQ // 128):
                        yps = psC.tile([128, 1024], F32, tag="y")
                        for sb in range(2):
                            for ci in range(CIN):
                                nc.tensor.matmul(
                                    yps[:, sb * 512:(sb + 1) * 512],
                                    ot_acc[:, ci, qt * 128:(qt + 1) * 128],
                                    wp[:, ci, sb * 512:(sb + 1) * 512],
                                    start=(ci == 0),
                                    stop=False,
                                )
                            nc.tensor.matmul(
                                yps[:, sb * 512:(sb + 1) * 512],
                                ones1,
                                bp_t[:, sb * 512:(sb + 1) * 512],
                                start=False,
                                stop=True,
                            )
                        yd = ydp.tile([128, 1024], F32, tag="yd")
                        nc.vector.tensor_copy(yd, yps)
                        nc.sync.dma_start(
                            out=y[qt * 128:(qt + 1) * 128, :], in_=yd
                        )
    nc.finalize()
    return nc


def _make_in_maps(inputs):
    bf = ml_dtypes.bfloat16
    x = np.asarray(inputs["x"], np.float32)
    wqkvt = np.ascontiguousarray(
        np.asarray(inputs["w_qkv"], np.float32).T).astype(bf)
    wprojt = np.ascontiguousarray(
        np.asarray(inputs["w_proj"], np.float32).T).astype(bf)
    bp = np.asarray(inputs["b_proj"], np.float32).reshape(1, DIM).astype(bf)
    weff = (np.asarray(inputs["sub_norm_w"], np.float32)
            * (1.0 - LAMBDA_INIT)).reshape(1, VD).astype(bf)
    in_maps = []
    for c in range(NCORES):
        b, half = c // 2, c % 2
        xt = np.asarray(x[b].T)  # [DIM, N]
        if half == 1:  # query rows first
            xt = np.concatenate([xt[:, NQ:], xt[:, :NQ]], axis=1)
        in_maps.append({
            "xbt": np.ascontiguousarray(xt).astype(bf),
            "wqkvt": wqkvt,
            "wprojt": wprojt,
            "bprojr": bp,
            "weffr": weff,
        })
    return in_maps


def kernel(x, w_qkv, w_proj, b_proj, lambda_q1, lambda_k1, lambda_q2,
           lambda_k2, sub_norm_w):
    lam = float(
        np.exp(np.sum(np.float64(lambda_q1) * np.float64(lambda_k1)))
        - np.exp(np.sum(np.float64(lambda_q2) * np.float64(lambda_k2)))
        + LAMBDA_INIT
    )
    key = round(lam, 12)
    if key not in _CACHE:
        _CACHE[key] = build_nc(lam)
    nc = _CACHE[key]

    in_maps = _make_in_maps({
        "x": x, "w_qkv": w_qkv, "w_proj": w_proj, "b_proj": b_proj,
        "sub_norm_w": sub_norm_w,
    })
    res = run_bass_kernel_spmd(nc, in_maps, list(range(NCORES)))
    out = np.empty((B, N, DIM), np.float32)
    for c in range(NCORES):
        b, half = c // 2, c % 2
        out[b, half * NQ:(half + 1) * NQ, :] = res.results[c]["y"]
    return out
